# revision 18
# baseline (speedup 1.0000x reference)
"""Double-centering kernel for Trainium2 (Bass/Tile), 8-core data parallel.

Computes T = -0.5 * (D - row_mean - col_mean + glob_mean) for
D: [256, 512, 512] f32, sharding the batch dim across 8 NeuronCores
(32 matrices per core, no cross-core communication).

Per-core layout: PAIRS of [512, 512] matrices are viewed as one
[128, 4096] SBUF tile (matrix m in cols m*2048..; partition p holds its
rows 4p..4p+3), so every DMA is one fully contiguous transfer.

bf16 dataflow (the 2e-2 harness tolerance allows it; measured rel err
~6e-3): loads cast f32->bf16 in the SWDGE DMA, on-chip compute is bf16
with f32 PSUM/accumulators for every reduction, the output is stored
bf16 (halving HBM write traffic) and upcast to f32 on the host.  HBM
per core: 32 MiB f32 read + 16 MiB bf16 write = 48 MiB.

All five engines carry ~equal per-pair work so the SWDGE load stream is
the pacer:
  A (it=bp):   GPSIMD load (SWDGE f32->bf16 cast); 2 of 8 v-chunks
                 (ts: v=-0.5*in, accum a)
               ACT   6 of 8 v-chunks (activation, accum a=-0.5*rowsum)
               PE    C0[m] += (ones/1024)^T @ chunk  (4-chunk PSUM accum
                       -> C0 = 0.5*col_mean, read directly by stt)
  B (it=bp+1): GPSIMD a16 = bf16(a); rowterm = (a-G)*(-1/512)
               PE    gs = (ones/512)^T @ a16         (PSUM)
               DVE   G[m] = sum_c gs  (= -256*gmean, grouped reduce)
               DVE   v_c = (v_c + rowterm_c) + C0[m] (stt, in place,
                       in1 streams straight from PSUM)
  C (it=bp+2): SP    store <- v  (HWDGE, bf16)
"""

from contextlib import ExitStack

import numpy as np

import concourse.bacc as bacc
import concourse.tile as tile
from concourse import mybir
from concourse.bass_utils import run_bass_kernel_spmd

N_CORES = 8
B = 256
N = 512
B_LOC = B // N_CORES  # 32 matrices per core
PAIR = 2
N_PAIRS = B_LOC // PAIR  # 16 DMA pairs per core
P = 128
CHUNKS = N // P  # 4
FREE = CHUNKS * N  # 2048 elems per partition per matrix
PFREE = PAIR * FREE  # 4096 per pair tile
LOOK = 7  # load lookahead (pairs)
GP_CHUNK = CHUNKS - 1  # per-matrix chunk index computed on gpsimd, not ACT

_COMPILED = None
LAST_RESULTS = None  # BassKernelResults of the most recent run (for test harness)


def _build():
    nc = bacc.Bacc("TRN2", target_bir_lowering=False, debug=False)
    d_in = nc.dram_tensor("d_in", [N_PAIRS, P, PFREE], mybir.dt.float32,
                          kind="ExternalInput")
    t_out = nc.dram_tensor("t_out", [N_PAIRS, P, PFREE], mybir.dt.bfloat16,
                           kind="ExternalOutput")
    f32 = mybir.dt.float32
    bf16 = mybir.dt.bfloat16

    with tile.TileContext(nc) as tc, ExitStack() as ctx:
        singles = ctx.enter_context(tc.tile_pool(name="singles", bufs=1))
        in_pool = ctx.enter_context(tc.tile_pool(name="in", bufs=N_PAIRS))
        v_pool = ctx.enter_context(tc.tile_pool(name="v", bufs=4))
        a_pool = ctx.enter_context(tc.tile_pool(name="a", bufs=3))
        a16_pool = ctx.enter_context(tc.tile_pool(name="a16", bufs=3))
        g_pool = ctx.enter_context(tc.tile_pool(name="g", bufs=3))
        rt_pool = ctx.enter_context(tc.tile_pool(name="rt", bufs=3))
        psum = ctx.enter_context(tc.tile_pool(name="psum", bufs=4, space="PSUM"))
        gs_pool = ctx.enter_context(tc.tile_pool(name="gs", bufs=2, space="PSUM"))

        ins = [None] * N_PAIRS

        # Pre-issue every load up front: the SWDGE (gpsimd) trigger queue
        # must never sit behind a data-dependent wait, or the load stream
        # throttles to the compute rhythm.
        for k in range(N_PAIRS):
            ins[k] = in_pool.tile([P, PFREE], bf16, name="in_t")
            nc.gpsimd.dma_start(out=ins[k][:], in_=d_in[k])

        # Stationary matrices fold the mean scales into the matmuls:
        # C0 = (ones/1024)^T @ D-chunks  -> 0.5*col_mean directly in PSUM;
        # gs = (ones/512)^T @ a16        -> per-chunk-position sums of a.
        ones_cs = singles.tile([P, P], bf16)
        nc.vector.memset(ones_cs[:], 1.0 / 1024.0)
        ones_gs = singles.tile([P, P], bf16)
        nc.vector.memset(ones_gs[:], -1.0 / 512.0)

        st = {}  # per-pair stage-A outputs carried to stage B
        for it in range(N_PAIRS + 2):
            # ---- stage B prologue pieces that must precede stage A work on
            # the same engines (a16 before gpsimd's chunk work, gs before
            # PE's next colsum block, G before DVE's stt).
            if 0 <= it - 1 < N_PAIRS:
                bq = it - 1
                v_b, c0s_b, a_b = st[bq]
                a16 = a16_pool.tile([P, PAIR * CHUNKS], bf16, name="a16")
                nc.gpsimd.tensor_scalar_mul(a16[:], a_b[:], 1.0)
                gs = gs_pool.tile([P, PAIR * CHUNKS], f32, name="gs")
                nc.tensor.matmul(out=gs[:], lhsT=ones_gs[:], rhs=a16[:],
                                 start=True, stop=True)
                G_b = g_pool.tile([P, PAIR], f32, name="G")
                nc.vector.tensor_reduce(
                    out=G_b[:], in_=gs[:].rearrange("p (m c) -> p m c", m=PAIR),
                    axis=mybir.AxisListType.X, op=mybir.AluOpType.add)

            if it < N_PAIRS:
                bp = it
                in_t = ins[bp]

                # Column means on PE: accumulate the 4 row-chunks of each
                # matrix through the scaled all-ones matmul into one PSUM
                # bank; C0 = 0.5*col_mean stays in PSUM for the stt.
                c0s = []
                for m in range(PAIR):
                    c0 = psum.tile([P, N], f32, name="c0")
                    for c in range(CHUNKS):
                        sl = slice(m * FREE + c * N, m * FREE + (c + 1) * N)
                        nc.tensor.matmul(out=c0[:], lhsT=ones_cs[:],
                                         rhs=in_t[:, sl], start=(c == 0),
                                         stop=(c == CHUNKS - 1))
                    c0s.append(c0)

                # v = -0.5*D (bf16); a_k = -0.5*rowsum(row 4p+c) in f32.
                # ACT owns chunks 0..2 of each matrix; DVE takes chunk 3 via
                # accumulating tensor_scalar (it fills DVE's G->rowterm
                # latency gap; Pool/gpsimd cannot run accumulating ops).
                v = v_pool.tile([P, PFREE], bf16, name="v")
                a = a_pool.tile([P, PAIR * CHUNKS], f32, name="a")
                for m in range(PAIR):
                    for c in range(CHUNKS):
                        sl = slice(m * FREE + c * N, m * FREE + (c + 1) * N)
                        k = m * CHUNKS + c
                        if c == GP_CHUNK:
                            nc.vector.tensor_scalar(
                                out=v[:, sl], in0=in_t[:, sl],
                                scalar1=-0.5, scalar2=0.0,
                                op0=mybir.AluOpType.mult,
                                op1=mybir.AluOpType.add,
                                accum_out=a[:, k:k + 1])
                        else:
                            nc.scalar.activation(
                                out=v[:, sl], in_=in_t[:, sl],
                                func=mybir.ActivationFunctionType.Copy,
                                bias=0.0, scale=-0.5,
                                accum_out=a[:, k:k + 1])
                st[bp] = (v, c0s, a)

            if 0 <= it - 1 < N_PAIRS:
                # ---- stage B main: rowterm on gpsimd, the fused final pass
                # on DVE reading C0 straight from PSUM.
                # rowterm = -(a + G)/512 = 0.5*row_mean - 0.5*glob_mean
                # (G = +256*gmean since ones_gs is negated).
                rowterm = rt_pool.tile([P, PAIR * CHUNKS], f32, name="rowterm")
                for m in range(PAIR):
                    ksl = slice(m * CHUNKS, (m + 1) * CHUNKS)
                    nc.gpsimd.tensor_scalar(out=rowterm[:, ksl],
                                            in0=a_b[:, ksl],
                                            scalar1=G_b[:, m:m + 1],
                                            scalar2=-1.0 / 512.0,
                                            op0=mybir.AluOpType.add,
                                            op1=mybir.AluOpType.mult)

                # out_c = (v_c + rowterm_c) + C0[m], fused, in place.
                for m in range(PAIR):
                    for c in range(CHUNKS):
                        sl = slice(m * FREE + c * N, m * FREE + (c + 1) * N)
                        k = m * CHUNKS + c
                        nc.vector.scalar_tensor_tensor(out=v_b[:, sl],
                                                       in0=v_b[:, sl],
                                                       scalar=rowterm[:, k:k + 1],
                                                       in1=c0s_b[m][:],
                                                       op0=mybir.AluOpType.add,
                                                       op1=mybir.AluOpType.add)

            if 0 <= it - 2 < N_PAIRS:
                br = it - 2
                nc.sync.dma_start(out=t_out[br], in_=st[br][0][:])

    nc.compile()
    return nc


def _get_nc():
    global _COMPILED
    if _COMPILED is None:
        _COMPILED = _build()
    return _COMPILED


def kernel(D: np.ndarray) -> np.ndarray:
    global LAST_RESULTS
    D = np.ascontiguousarray(np.asarray(D), dtype=np.float32)
    assert D.shape == (B, N, N), D.shape
    shards = D.reshape(N_CORES, N_PAIRS, PAIR, P, FREE)
    # pair tile layout: [128, 2*2048] with matrix m at cols m*2048..
    shards = shards.transpose(0, 1, 3, 2, 4).reshape(N_CORES, N_PAIRS, P, PFREE)
    nc = _get_nc()
    in_maps = [{"d_in": np.ascontiguousarray(shards[i])} for i in range(N_CORES)]
    res = run_bass_kernel_spmd(nc, in_maps, core_ids=list(range(N_CORES)))
    LAST_RESULTS = res
    out = np.stack([np.asarray(res.results[i]["t_out"]).astype(np.float32)
                    for i in range(N_CORES)])
    out = out.reshape(N_CORES, N_PAIRS, P, PAIR, FREE).transpose(0, 1, 3, 2, 4)
    return np.ascontiguousarray(out).reshape(B, N, N)


# revision 19
# speedup vs baseline: 1.0829x; 1.0829x over previous
"""Double-centering kernel for Trainium2 (Bass/Tile), 8-core data parallel.

Computes T = -0.5 * (D - row_mean - col_mean + glob_mean) for
D: [256, 512, 512] f32, sharding the batch dim across 8 NeuronCores
(32 matrices per core, no cross-core communication).

Per-core layout: PAIRS of [512, 512] matrices are viewed as one
[128, 4096] SBUF tile (matrix m in cols m*2048..; partition p holds its
rows 4p..4p+3), so every DMA is one fully contiguous transfer.

bf16 dataflow (the 2e-2 harness tolerance allows it; measured rel err
5.6e-3): loads cast f32->bf16 in the SWDGE DMA, on-chip compute is bf16
with f32 PSUM/accumulators for every reduction, the output is stored
bf16 (halving HBM write traffic) and upcast to f32 on the host.  HBM
per core: 32 MiB f32 read + 16 MiB bf16 write = 48 MiB.

All 16 SWDGE loads are pre-issued before any compute so the trigger
queue never sits behind a data-dependent wait.  Per-pair work is spread
so no engine exceeds ~8 us:
  A (it=bp):   ACT   6 of 8 v-chunks: v=-0.5*in, accum a=-0.5*rowsum
               DVE   2 of 8 v-chunks (accumulating tensor_scalar)
               PE    C0[m] += ones^T @ chunk   (4-chunk PSUM accum)
  B (it=bp+1): ACT   csc[m] = C0/1024 (bf16; accum gsum = 256*gmean)
               GPSIMD rowterm = -(a + gsum)/512 (= .5row_mean-.5gmean)
               DVE   v_c = (v_c + rowterm_c) + csc[m]  (stt, in place)
  C (it=bp+2): SP    store <- v  (HWDGE, bf16)
"""

from contextlib import ExitStack

import numpy as np

import concourse.bacc as bacc
import concourse.tile as tile
from concourse import mybir
from concourse.bass_utils import run_bass_kernel_spmd

N_CORES = 8
B = 256
N = 512
B_LOC = B // N_CORES  # 32 matrices per core
PAIR = 2
N_PAIRS = B_LOC // PAIR  # 16 DMA pairs per core
P = 128
CHUNKS = N // P  # 4
FREE = CHUNKS * N  # 2048 elems per partition per matrix
PFREE = PAIR * FREE  # 4096 per pair tile
DVE_CHUNK = CHUNKS - 1  # per-matrix chunk index computed on DVE, not ACT

_COMPILED = None
LAST_RESULTS = None  # BassKernelResults of the most recent run (for test harness)


def _build():
    nc = bacc.Bacc("TRN2", target_bir_lowering=False, debug=False)
    d_in = nc.dram_tensor("d_in", [N_PAIRS, P, PFREE], mybir.dt.float32,
                          kind="ExternalInput")
    t_out = nc.dram_tensor("t_out", [N_PAIRS, P, PFREE], mybir.dt.bfloat16,
                           kind="ExternalOutput")
    f32 = mybir.dt.float32
    bf16 = mybir.dt.bfloat16

    with tile.TileContext(nc) as tc, ExitStack() as ctx:
        singles = ctx.enter_context(tc.tile_pool(name="singles", bufs=1))
        in_pool = ctx.enter_context(tc.tile_pool(name="in", bufs=N_PAIRS))
        v_pool = ctx.enter_context(tc.tile_pool(name="v", bufs=4))
        csc_pool = ctx.enter_context(tc.tile_pool(name="csc", bufs=3))
        a_pool = ctx.enter_context(tc.tile_pool(name="a", bufs=3))
        g_pool = ctx.enter_context(tc.tile_pool(name="g", bufs=3))
        rt_pool = ctx.enter_context(tc.tile_pool(name="rt", bufs=3))
        psum = ctx.enter_context(tc.tile_pool(name="psum", bufs=4, space="PSUM"))

        ins = [None] * N_PAIRS
        # Pre-issue every load up front: the SWDGE (gpsimd) trigger queue
        # must never sit behind a data-dependent wait, or the load stream
        # throttles to the compute rhythm.
        for k in range(N_PAIRS):
            ins[k] = in_pool.tile([P, PFREE], bf16, name="in_t")
            nc.gpsimd.dma_start(out=ins[k][:], in_=d_in[k])

        # ones/1024 stationary: C0 = 0.5*col_mean lands directly in PSUM.
        ones_cs = singles.tile([P, P], bf16)
        nc.vector.memset(ones_cs[:], 1.0 / 1024.0)

        st = {}  # per-pair stage-A outputs carried to stage B
        for it in range(N_PAIRS + 2):
            if it < N_PAIRS:
                bp = it
                in_t = ins[bp]

                # Column means on PE: accumulate the 4 row-chunks of each
                # matrix through the scaled all-ones matmul into one PSUM
                # bank; csc reads C0 = 0.5*col_mean from there.
                c0s = []
                for m in range(PAIR):
                    c0 = psum.tile([P, N], f32, name="c0")
                    for c in range(CHUNKS):
                        sl = slice(m * FREE + c * N, m * FREE + (c + 1) * N)
                        nc.tensor.matmul(out=c0[:], lhsT=ones_cs[:],
                                         rhs=in_t[:, sl], start=(c == 0),
                                         stop=(c == CHUNKS - 1))
                    c0s.append(c0)

                # v = -0.5*D (bf16); a_k = -0.5*rowsum(row 4p+c) in f32.
                # ACT owns chunks 0..2 of each matrix, DVE chunk 3 (keeps
                # both engines under ~8 us/pair).
                v = v_pool.tile([P, PFREE], bf16, name="v")
                a = a_pool.tile([P, PAIR * CHUNKS], f32, name="a")
                for m in range(PAIR):
                    for c in range(CHUNKS):
                        sl = slice(m * FREE + c * N, m * FREE + (c + 1) * N)
                        k = m * CHUNKS + c
                        if c == DVE_CHUNK:
                            nc.vector.tensor_scalar(
                                out=v[:, sl], in0=in_t[:, sl],
                                scalar1=-0.5, scalar2=0.0,
                                op0=mybir.AluOpType.mult,
                                op1=mybir.AluOpType.add,
                                accum_out=a[:, k:k + 1])
                        else:
                            nc.scalar.activation(
                                out=v[:, sl], in_=in_t[:, sl],
                                func=mybir.ActivationFunctionType.Copy,
                                bias=0.0, scale=-0.5,
                                accum_out=a[:, k:k + 1])
                st[bp] = (v, c0s, a)

            if 0 <= it - 1 < N_PAIRS:
                bq = it - 1
                v_b, c0s_b, a_b = st[bq]

                # csc = 0.5*col_mean (bf16 in SBUF: the stt reads it 4x, and
                # bf16 SBUF reads beat f32 PSUM reads); gsum = 256*glob_mean.
                csc = csc_pool.tile([P, PAIR, N], bf16, name="csc")
                gsum = g_pool.tile([P, PAIR], f32, name="gsum")
                for m in range(PAIR):
                    nc.scalar.activation(out=csc[:, m, :], in_=c0s_b[m][:],
                                         func=mybir.ActivationFunctionType.Copy,
                                         bias=0.0, scale=1.0,
                                         accum_out=gsum[:, m:m + 1])

                # rowterm = -(a + gsum)/512 = 0.5*row_mean - 0.5*glob_mean.
                rowterm = rt_pool.tile([P, PAIR * CHUNKS], f32, name="rowterm")
                for m in range(PAIR):
                    ksl = slice(m * CHUNKS, (m + 1) * CHUNKS)
                    nc.gpsimd.tensor_scalar(out=rowterm[:, ksl],
                                            in0=a_b[:, ksl],
                                            scalar1=gsum[:, m:m + 1],
                                            scalar2=-1.0 / 512.0,
                                            op0=mybir.AluOpType.add,
                                            op1=mybir.AluOpType.mult)

                # out_c = (v_c + rowterm_c) + csc[m], fused, in place.
                for m in range(PAIR):
                    for c in range(CHUNKS):
                        sl = slice(m * FREE + c * N, m * FREE + (c + 1) * N)
                        k = m * CHUNKS + c
                        nc.vector.scalar_tensor_tensor(out=v_b[:, sl],
                                                       in0=v_b[:, sl],
                                                       scalar=rowterm[:, k:k + 1],
                                                       in1=csc[:, m, :],
                                                       op0=mybir.AluOpType.add,
                                                       op1=mybir.AluOpType.add)

            if 0 <= it - 2 < N_PAIRS:
                br = it - 2
                nc.sync.dma_start(out=t_out[br], in_=st[br][0][:])

    nc.compile()
    return nc


def _get_nc():
    global _COMPILED
    if _COMPILED is None:
        _COMPILED = _build()
    return _COMPILED


def kernel(D: np.ndarray) -> np.ndarray:
    global LAST_RESULTS
    D = np.ascontiguousarray(np.asarray(D), dtype=np.float32)
    assert D.shape == (B, N, N), D.shape
    shards = D.reshape(N_CORES, N_PAIRS, PAIR, P, FREE)
    # pair tile layout: [128, 2*2048] with matrix m at cols m*2048..
    shards = shards.transpose(0, 1, 3, 2, 4).reshape(N_CORES, N_PAIRS, P, PFREE)
    nc = _get_nc()
    in_maps = [{"d_in": np.ascontiguousarray(shards[i])} for i in range(N_CORES)]
    res = run_bass_kernel_spmd(nc, in_maps, core_ids=list(range(N_CORES)))
    LAST_RESULTS = res
    out = np.stack([np.asarray(res.results[i]["t_out"]).astype(np.float32)
                    for i in range(N_CORES)])
    out = out.reshape(N_CORES, N_PAIRS, P, PAIR, FREE).transpose(0, 1, 3, 2, 4)
    return np.ascontiguousarray(out).reshape(B, N, N)
